# revision 13
# baseline (speedup 1.0000x reference)
"""Trainium2 Bass kernel for nn_CriticNetwork (LSTM T=3, D=18, H=64 + MLP 64->32->1).

v2: pure data parallel over 8 NeuronCores (65536 batch each), 64 iters of
1024 elements (two 512-wide sub-tiles A/B stacked on partitions 0:64/64:128).

Key layout: block-diagonal lhs weights compute each gate for A and B halves
in ONE matmul ([128,N] out = [gate_A; gate_B]); h_t tiles are consumed
directly as rhs of the next step's K=128 block-diag h-matmul (no staging).
Per (iter,step) a 3-bank PSUM tile [g|i|x] (x = o or f) is drained by one
wide sigmoid on ScalarE ([i|x]) and a fused deg-5 poly custom DVE op on g:
  PTMUL5: out = in1 * tanh5(in0)   (ig product, h=o*tanh(c))
  SIGP5:  out = sig5(in0)          (o-gate sigma via odd poly + 0.5)
f*c and c=p+q run on Pool. MLP z/v matmuls share one PSUM bank; v rows are
DMA'd straight from PSUM. Stages are emitted with a 4-deep iteration skew so
all five engines pipeline across adjacent iterations.
"""
import os
import numpy as np
import ml_dtypes

import concourse.bacc as bacc
import concourse.bass as bass
import concourse.mybir as mybir
import concourse.tile as tile
from concourse import bass_utils

F32 = mybir.dt.float32
BF16 = mybir.dt.bfloat16
AF = mybir.ActivationFunctionType

NCORES = 8
W = 512                         # sub-tile width (psum bank, f32)
ITERS = int(os.environ.get("K_ITERS", "64"))
BCORE = ITERS * 2 * W           # 65536 at full size
BLK = 8                         # iters per DMA block
NBLK = (ITERS + BLK - 1) // BLK

STATE_DIM, SEQ_LEN, HIDDEN, MLP_HIDDEN = 18, 3, 64, 32
KXR = 2 * (STATE_DIM + 2)       # real stacked x rows: [x_A;1;1;x_B;1;1] = 40
KX = 96                         # padded to 96: K>=96 matmuls stream at full
                                # PE clock (2.4GHz); K<=84 run at half speed.
                                # lhs rows 40:96 are zero; rhs rows 40:96 hold
                                # duplicated (finite) x data.

# deg-5 odd poly coeffs (density-weighted lstsq fits, see _coeffs deriv):
# tanh(x) on |x|<=2.7 (gate g), tanh(x) on |x|<=1.4 (cell c),
# sigma(x)-0.5 on |x|<=2.7 (gate o)
T5G = (0.97818024, -0.22805961, 0.02183675)
T5C = (0.99800597, -0.30731648, 0.06561998)
S5 = (0.24966609, -0.0194509, 0.00106699)

_ops = None


def get_ops():
    """Register PTMUL5 (in1*poly5(in0)) and SIGP5 (poly5(in0)+C3) DVE ops."""
    global _ops
    if _ops is not None:
        return _ops
    import concourse.dve_ops as dve_ops
    from concourse.dve_spec import (Spec, Src0, Src1, C0, C1, C2, C3, sq,
                                    lower, _spill_c3_to_src1)
    from concourse.dve_uop import DveOpSpec

    def _reg(name, body, ref):
        for op in dve_ops.OPS:
            if op.name == name:
                return op
        spec = Spec(body=body, reference=ref)
        if name not in dve_ops._SUB_OPCODE_FOR_NAME:
            dve_ops._SUB_OPCODE_FOR_NAME[name] = (
                max(dve_ops._SUB_OPCODE_FOR_NAME.values()) + 1)
        shas = {}
        for ver in ("v3", "v4"):
            try:
                s = DveOpSpec(name=name, opcode=dve_ops.get_dve_sub_opcode(name),
                              uops=lower(spec, ver=ver), rd1_en=True)
                shas[ver] = s.sha(ver)
            except Exception:
                pass
        op = dve_ops.DveOp(name, spec, subdim=False, uops_sha=shas)
        dve_ops.OPS.append(op)
        return op

    u = sq(Src0)
    pt_body = (((C2 * u + C1) * u + C0) * Src0) * Src1

    def _pt_ref(in0, in1, s0, s1, imm2):
        x = in0.astype(np.float32)
        uu = x * x
        return ((imm2 * uu + s1) * uu + s0) * x * in1.astype(np.float32)

    sg_body = _spill_c3_to_src1((((C2 * u + C1) * u + C0) * Src0) + C3)

    def _sg_ref(in0, in1, s0, s1, imm2):
        x = in0.astype(np.float32)
        uu = x * x
        c3 = np.asarray(in1, np.float32).reshape(in1.shape[0], 1)
        return ((imm2 * uu + s1) * uu + s0) * x + c3

    _ops = (_reg("PTMUL5_ANT", pt_body, _pt_ref),
            _reg("SIGP5_ANT", sg_body, _sg_ref))
    return _ops


# gate column blocks in hlhs/xlhs: [i f o g]
GC = {"i": 0, "f": 128, "o": 256, "g": 384}


def build_bass():
    ptmul5, sigp5 = get_ops()
    nc = bacc.Bacc("TRN2", target_bir_lowering=False, debug=False)

    xt_d = [nc.dram_tensor(f"xt{t}", [KX, BCORE // 2], BF16,
                           kind="ExternalInput").ap() for t in range(SEQ_LEN)]
    hlhs_d = nc.dram_tensor("hlhs", [128, 512], BF16, kind="ExternalInput").ap()
    xlhs_d = nc.dram_tensor("xlhs", [KX, 512], BF16, kind="ExternalInput").ap()
    zlhs_d = nc.dram_tensor("zlhs", [128, 64], BF16, kind="ExternalInput").ap()
    vlhs_d = nc.dram_tensor("vlhs", [128, 2], BF16, kind="ExternalInput").ap()
    b1_d = nc.dram_tensor("b1r", [128, 1], F32, kind="ExternalInput").ap()
    out_d = nc.dram_tensor("out", [2 * ITERS, W], F32, kind="ExternalOutput").ap()

    BW = BLK * W  # dma block width (4096) in xt cols

    with tile.TileContext(nc) as tc:
        with tc.tile_pool(name="const", bufs=1) as cpool, \
             tc.tile_pool(name="x0", bufs=3) as x0p, \
             tc.tile_pool(name="x1", bufs=3) as x1p, \
             tc.tile_pool(name="x2", bufs=3) as x2p, \
             tc.tile_pool(name="sig", bufs=6) as sigp, \
             tc.tile_pool(name="tos", bufs=5) as tosp, \
             tc.tile_pool(name="cw", bufs=6) as cwp, \
             tc.tile_pool(name="hw", bufs=6) as hwp, \
             tc.tile_pool(name="pq", bufs=6) as pqp, \
             tc.tile_pool(name="rz", bufs=4) as rzp, \
             tc.tile_pool(name="vc", bufs=4) as vcp, \
             tc.tile_pool(name="p3", bufs=2, space="PSUM") as p3p, \
             tc.tile_pool(name="os", bufs=1, space="PSUM") as osp, \
             tc.tile_pool(name="zv", bufs=1, space="PSUM") as zvp:

            hlhs = cpool.tile([128, 512], BF16)
            nc.sync.dma_start(hlhs[:], hlhs_d[:])
            xlhs = cpool.tile([KX, 512], BF16)
            nc.sync.dma_start(xlhs[:], xlhs_d[:])
            zlhs = cpool.tile([128, 64], BF16)
            nc.sync.dma_start(zlhs[:], zlhs_d[:])
            vlhs = cpool.tile([128, 2], BF16)
            nc.sync.dma_start(vlhs[:], vlhs_d[:])
            b1r = cpool.tile([128, 1], F32)
            nc.sync.dma_start(b1r[:], b1_d[:])
            halft = cpool.tile([128, 1], F32)
            nc.vector.memset(halft[:], 0.5)
            zv = zvp.tile([128, W], F32, tag="zv")
            # rows 0:64 of zv feed the K=128 v-matmul rhs via relu (zero
            # weights) — init once so no NaN/Inf garbage propagates
            nc.vector.memset(zv[0:64, :], 0.0)

            xts = {}    # (t, blk) -> tile
            st = {}     # per-iter state tiles

            def xcol(t, j):
                b, r = j // BLK, j % BLK
                return xts[(t, b)][:, r * W:(r + 1) * W]

            def gate_pair(psl, gn, hrhs, xrhs):
                """h-part (K=128 blockdiag) + x-part (K=40) accumulated."""
                nc.tensor.matmul(psl, hlhs[:, GC[gn]:GC[gn] + 128], hrhs,
                                 start=True, stop=False, skip_group_check=True)
                nc.tensor.matmul(psl, xlhs[:, GC[gn]:GC[gn] + 128], xrhs,
                                 start=False, stop=True, skip_group_check=True)

            def emit_s1(j):
                t = p3p.tile([128, 3 * W], F32, tag="p3")   # [g|i|o]
                x = xcol(0, j)
                nc.tensor.matmul(t[:, 0:W], xlhs[:, GC["g"]:GC["g"] + 128], x,
                                 start=True, stop=True, skip_group_check=True)
                nc.tensor.matmul(t[:, W:2 * W], xlhs[:, GC["i"]:GC["i"] + 128],
                                 x, start=True, stop=True, skip_group_check=True)
                nc.tensor.matmul(t[:, 2 * W:3 * W], xlhs[:, GC["o"]:GC["o"] + 128],
                                 x, start=True, stop=True, skip_group_check=True)
                tio = sigp.tile([128, 2 * W], BF16, tag="sig")
                nc.scalar.activation(tio[:], t[:, W:3 * W], AF.Sigmoid)
                c1 = cwp.tile([128, W], BF16, tag="c")
                nc.vector._custom_dve(ptmul5, out=c1[:], in0=t[:, 0:W],
                                      in1=tio[:, 0:W],
                                      s0=T5G[0], s1=T5G[1], imm2=T5G[2])
                h1 = hwp.tile([128, W], BF16, tag="h")
                nc.vector._custom_dve(ptmul5, out=h1[:], in0=c1[:],
                                      in1=tio[:, W:2 * W],
                                      s0=T5C[0], s1=T5C[1], imm2=T5C[2])
                st[("c", j)] = c1
                st[("h", j)] = h1

            def emit_s23(j, step):
                x = xcol(step - 1, j)
                hprev = st[("h", j)]
                cprev = st[("c", j)]
                t = p3p.tile([128, 3 * W], F32, tag="p3")   # [g|i|f]
                gate_pair(t[:, 0:W], "g", hprev[:], x)
                gate_pair(t[:, W:2 * W], "i", hprev[:], x)
                gate_pair(t[:, 2 * W:3 * W], "f", hprev[:], x)
                po = osp.tile([128, W], F32, tag="os")
                gate_pair(po[:], "o", hprev[:], x)

                tif = sigp.tile([128, 2 * W], BF16, tag="sig")
                nc.scalar.activation(tif[:], t[:, W:3 * W], AF.Sigmoid)
                to = tosp.tile([128, W], BF16, tag="to")
                if step == 2:
                    nc.scalar.activation(to[:], po[:], AF.Sigmoid)
                else:
                    nc.vector._custom_dve(sigp5, out=to[:], in0=po[:],
                                          in1=halft[:],
                                          s0=S5[0], s1=S5[1], imm2=S5[2])
                p = pqp.tile([128, W], BF16, tag="pq")
                nc.vector._custom_dve(ptmul5, out=p[:], in0=t[:, 0:W],
                                      in1=tif[:, 0:W],
                                      s0=T5G[0], s1=T5G[1], imm2=T5G[2])
                q = pqp.tile([128, W], BF16, tag="pq")
                nc.gpsimd.tensor_mul(q[:], tif[:, W:2 * W], cprev[:])
                c = cwp.tile([128, W], BF16, tag="c")
                nc.gpsimd.tensor_add(c[:], p[:], q[:])
                h = hwp.tile([128, W], BF16, tag="h")
                nc.vector._custom_dve(ptmul5, out=h[:], in0=c[:], in1=to[:],
                                      s0=T5C[0], s1=T5C[1], imm2=T5C[2])
                st[("c", j)] = c
                st[("h", j)] = h

            def emit_mlp(j):
                # z lives at psum rows 64:128 of zv; v(even j) at rows 0:2,
                # v(odd j) at rows 32:34. Per j-pair one scalar Identity
                # drains rows 0:34 to SBUF, then two DMAs write out.
                h3 = st.pop(("h", j))
                st.pop(("c", j))
                nc.tensor.matmul(zv[64:128, :], zlhs[:], h3[:],
                                 start=True, stop=True, skip_group_check=True)
                rz = rzp.tile([128, W], BF16, tag="rz")
                nc.scalar.activation(rz[:], zv[:], AF.Relu, bias=b1r[:])
                r0 = 32 * (j % 2)
                nc.tensor.matmul(zv[r0:r0 + 2, :], vlhs[:], rz[:],
                                 start=True, stop=True,
                                 tile_position=(0, r0), skip_group_check=True)
                if j % 2 == 1:
                    vc = vcp.tile([34, W], F32, tag="vc")
                    nc.scalar.activation(vc[:], zv[0:34, :], AF.Identity)
                    nc.sync.dma_start(out_d[2 * j - 2:2 * j, :], vc[0:2, :])
                    nc.sync.dma_start(out_d[2 * j:2 * j + 2, :], vc[32:34, :])

            # Round emit order [s3, s2, s1, mlp]: the p3 pool holds 2 bufs for
            # 3 allocs/round, so s1(r) reuses s3(r-2)'s buffer — emitting s3
            # first puts that buffer's drains (sigma3, PT5G, SIG5) at the head
            # of every engine queue, ahead of s1's late-round matmuls.
            pools = {0: x0p, 1: x1p, 2: x2p}
            for r in range(ITERS + 3):
                if r < ITERS and r % BLK == 0:
                    b = r // BLK
                    for t in range(SEQ_LEN):
                        xb = pools[t].tile([KX, BW], BF16, tag=f"x{t}")
                        nc.sync.dma_start(
                            xb[:], xt_d[t][:, b * BW:(b + 1) * BW])
                        xts[(t, b)] = xb
                if 2 <= r < ITERS + 2:
                    emit_s23(r - 2, 3)
                if 1 <= r < ITERS + 1:
                    emit_s23(r - 1, 2)
                if r < ITERS:
                    emit_s1(r)
                if 3 <= r:
                    emit_mlp(r - 3)

    nc.compile()
    return nc


def _host_prep(state_seq, W_ih, W_hh, b_ih, b_hh, W1, b1, W2, b2):
    """Build per-core input maps (host-side layout prep only)."""
    bfd = ml_dtypes.bfloat16
    B = state_seq.shape[0]
    H = HIDDEN
    b = b_ih.astype(np.float64) + b_hh.astype(np.float64)

    # reference gate order: i, f, g, o
    Wx = {g: W_ih[k * H:(k + 1) * H].astype(np.float64)
          for k, g in enumerate("ifgo")}
    Wh = {g: W_hh[k * H:(k + 1) * H].astype(np.float64)
          for k, g in enumerate("ifgo")}
    bg = {g: b[k * H:(k + 1) * H] for k, g in enumerate("ifgo")}

    hlhs = np.zeros((128, 512), np.float64)
    xlhs = np.zeros((KX, 512), np.float64)
    for g, c0 in GC.items():
        hlhs[0:64, c0:c0 + 64] = Wh[g].T
        hlhs[64:128, c0 + 64:c0 + 128] = Wh[g].T
        bhi = bg[g].astype(bfd).astype(np.float64)
        blo = bg[g] - bhi
        for blk, r0 in ((0, 0), (1, 20)):
            cc = c0 + 64 * blk
            xlhs[r0:r0 + 18, cc:cc + 64] = Wx[g].T
            xlhs[r0 + 18, cc:cc + 64] = bhi
            xlhs[r0 + 19, cc:cc + 64] = blo

    zlhs = np.zeros((128, 64), np.float64)
    zlhs[0:64, 0:32] = W1.astype(np.float64).T
    zlhs[64:128, 32:64] = W1.astype(np.float64).T
    vlhs = np.zeros((128, 2), np.float64)
    vlhs[64:96, 0] = W2[0].astype(np.float64)
    vlhs[96:128, 1] = W2[0].astype(np.float64)
    b1r = np.zeros((128, 1), np.float32)
    b1r[64:96, 0] = b1
    b1r[96:128, 0] = b1

    shared = {
        "hlhs": hlhs.astype(bfd), "xlhs": xlhs.astype(bfd),
        "zlhs": zlhs.astype(bfd), "vlhs": vlhs.astype(bfd), "b1r": b1r,
    }

    # xt layout: [40, BCORE/2] per step; iter j cols 512j:512j+512,
    # rows [x_A.T;1;1;x_B.T;1;1] where A/B are the iter's 512-halves.
    xs = state_seq.astype(bfd).astype(np.float32)  # quantize once
    in_maps = []
    for cc in range(NCORES):
        lo = cc * BCORE
        m = dict(shared)
        for t in range(SEQ_LEN):
            xt = np.ones((KX, BCORE // 2), np.float32)
            xc = xs[lo:lo + BCORE, t, :]            # [BCORE, 18]
            xc = xc.reshape(ITERS, 2, W, STATE_DIM)  # [j, half, col, d]
            xA = xc[:, 0].transpose(2, 0, 1).reshape(STATE_DIM, -1)
            xB = xc[:, 1].transpose(2, 0, 1).reshape(STATE_DIM, -1)
            xt[0:18, :] = xA
            xt[20:38, :] = xB
            # rows 40:96: zero-weight padding rows (K>=96 full-speed rule);
            # fill with finite duplicate data so 0*x never becomes NaN
            xt[40:80, :] = xt[0:40, :]
            xt[80:96, :] = xt[0:16, :]
            m[f"xt{t}"] = xt.astype(bfd)
        in_maps.append(m)
    return in_maps


_cached = {}


def kernel(**inputs) -> np.ndarray:
    if "nc" not in _cached:
        _cached["nc"] = build_bass()
    nc = _cached["nc"]
    in_maps = _host_prep(**inputs)
    trace = bool(int(os.environ.get("K_TRACE", "0")))
    res = bass_utils.run_bass_kernel_spmd(nc, in_maps, core_ids=list(range(NCORES)),
                                          trace=trace)
    b2 = np.float32(inputs["b2"][0])
    outs = [r["out"].reshape(-1).astype(np.float32) + b2 for r in res.results]
    _cached["last_results"] = res
    return np.concatenate(outs)
